# revision 1
# baseline (speedup 1.0000x reference)
"""Trainium2 Bass kernel for NeighborCompressedNN (retrieval kNN + gated MLP).

Query-parallel over 8 NeuronCores (no collectives): each core owns 128 of the
1024 queries and scans the full database.

Phase 1 (scan), per 4096-column group:
  - fp16 matmul (1 cyc/row on PE vs 4 for fp32): score s = x.X - ||X||^2/2
    via an augmented K=66 contraction (64 features + norm_hi + norm_lo fp16
    split so the norm bias keeps ~fp32 accuracy).
  - ACT copies PSUM fp32 -> SBUF fp16 score tiles (the 0.83ns/col wall).
  - DVE tensor-tensor MAX tree (fp16 2x mode, 0.52ns/col) folds each group
    8:1 into a 512-wide coset blockmax bm[b] = max_j s[b + 512j], then one
    max8 + max_index per 8192-wide pair keeps 8 candidates. This replaces
    the baseline's fp32 max8+max_index full passes (2.08 -> ~0.77 ns/col).
Phase 2 (merge): candidates fp16->fp32 (low 13 mantissa bits zero); the slot
  id is OR-ed into the low bits, making values unique — fp16 score ties no
  longer produce duplicate winners — and 4 rounds of max8 + match_replace
  give the top-32 with slots decoded from the value bits.
Phase 3 (rescore): winner slot -> coset-block id via scalar_tensor_tensor
  extraction; one single-offset indirect DMA per winner fetches its 8
  candidate rows (host prearranged XrC so each coset is contiguous); DVE
  rescores the 8 rows in fp16 and selects the argmax row with an exclusive
  mask + fold (no second gather).
Phase 4 (head): PE-transpose selected rows, fp16 gate matmul + tanh,
  neighbor-sum, 2-layer MLP head in fp32, sigmoid.

kernel(**inputs) takes the full unsharded inputs, shards queries across the
8 cores on the host, and returns the full [1024, 1] output.
"""

import numpy as np

import concourse.bass as bass
import concourse.mybir as mybir
import concourse.tile as tile
from concourse import bacc
from concourse.bass import ds, ts
from concourse.masks import make_identity

F32 = mybir.dt.float32
F16 = mybir.dt.float16
U32 = mybir.dt.uint32
U16 = mybir.dt.uint16
I32 = mybir.dt.int32

B, N, F = 1024, 200000, 64
K = 32
C, H = 16, 128
CORES = 8
QPC = B // CORES
P = 128

GRP = 4096
NG = (N + GRP - 1) // GRP   # 49
NPAD = NG * GRP             # 200704
COSET = 512
NPAIR = (NG + 1) // 2       # 25 selection units (8192 wide; last is 4096)
NCAND = NPAIR * 8           # 200
KF = F + 2
FW = F + 2
NEG = -3.0e38
PADV = -50000.0
GP_EVERY = 5                # every GP_EVERY-th half-copy goes to GPSIMD
NBLK = NPAD // 8            # 25088 coset blocks of 8 rows
FWG = 68                    # block row: [X(64) | norm | y | 0 | 0]


def build_program(loop_reps=1, num_devices=CORES):
    nc = bacc.Bacc(
        "TRN2",
        target_bir_lowering=False,
        debug=False,
        enable_asserts=False,
        num_devices=num_devices,
    )

    xT = nc.dram_tensor("xT", [KF, QPC], F16, kind="ExternalInput").ap()
    xT32 = nc.dram_tensor("xT32", [F, QPC], F32, kind="ExternalInput").ap()
    xRe = nc.dram_tensor("xRe", [QPC, FWG], F16, kind="ExternalInput").ap()
    XtA = nc.dram_tensor("XtA", [KF, NPAD], F16, kind="ExternalInput").ap()
    XrC = nc.dram_tensor("XrC", [NBLK, 8 * FWG], F16,
                         kind="ExternalInput").ap()
    Wg = nc.dram_tensor("Wg", [FWG, C], F16, kind="ExternalInput").ap()
    W1 = nc.dram_tensor("W1", [F + C, H], F32, kind="ExternalInput").ap()
    Wl = nc.dram_tensor("Wl", [H, 1], F32, kind="ExternalInput").ap()
    bg = nc.dram_tensor("bg", [C, 1], F32, kind="ExternalInput").ap()
    b1 = nc.dram_tensor("b1", [H, 1], F32, kind="ExternalInput").ap()
    bl = nc.dram_tensor("bl", [1, 1], F32, kind="ExternalInput").ap()

    out = nc.dram_tensor("out", [1, QPC], F32, kind="ExternalOutput").ap()

    with tile.TileContext(nc) as tc:
        with tc.tile_pool(name="const", bufs=1) as const:
            # only xT is needed before the scan; other DMA const loads are
            # deferred past the first rhs loads to cut HWDGE startup serial
            xT_t = const.tile([KF, QPC], F16)
            nc.sync.dma_start(xT_t[:], xT)
            xT32_t = const.tile([F, QPC], F32)
            xRe_t = const.tile([QPC, FWG], F16)
            Wg_t = const.tile([FWG, C], F16)
            W1_t = const.tile([F + C, H], F32)
            Wl_t = const.tile([H, 1], F32)
            bg_t = const.tile([C, 1], F32)
            b1_t = const.tile([H, 1], F32)
            bl_t = const.tile([1, 1], F32)
            ident = const.tile([P, P], F32)
            make_identity(nc, ident[:])


            slot_u = const.tile([P, NCAND], U32)
            nc.gpsimd.iota(slot_u[:], pattern=[[1, NCAND]], base=0,
                           channel_multiplier=0)
            slotf = const.tile([P, NCAND], F32)
            nc.vector.tensor_copy(slotf[:], slot_u[:])
            gblk_u = const.tile([P, NCAND], U32)     # slot -> block base
            nc.gpsimd.iota(gblk_u[:], pattern=[[2 * COSET, NPAIR], [0, 8]],
                           base=0, channel_multiplier=0)
            gblkf = const.tile([P, NCAND], F32)
            nc.vector.tensor_copy(gblkf[:], gblk_u[:])
            mask9 = const.tile([P, 8], U32)
            nc.gpsimd.iota(mask9[:], pattern=[[0, 8]], base=0x1FF,
                           channel_multiplier=0)
            jc_u = const.tile([P, 8, 8], U32)        # 1..8, j innermost
            nc.gpsimd.iota(jc_u[:], pattern=[[0, 8], [1, 8]], base=1,
                           channel_multiplier=0)
            jconst = const.tile([P, 8, 8], F32)
            nc.vector.tensor_copy(jconst[:], jc_u[:])
            ident16 = const.tile([P, P], F16)
            nc.vector.tensor_copy(ident16[:], ident[:])

            cand_v16 = const.tile([P, NCAND], F16)
            cand_pos = const.tile([P, NCAND], U16)

            def load_tail_consts():
                nc.sync.dma_start(xT32_t[:], xT32)
                nc.sync.dma_start(xRe_t[:], xRe)
                nc.sync.dma_start(Wg_t[:], Wg)
                nc.sync.dma_start(W1_t[:], W1)
                nc.sync.dma_start(Wl_t[:], Wl)
                nc.sync.dma_start(bg_t[:], bg)
                nc.sync.dma_start(b1_t[:], b1)
                nc.sync.dma_start(bl_t[:], bl)

            # ---- phase 1: scan (group pairs) ----
            half_ctr = 0
            with (
                tc.tile_pool(name="rhs", bufs=3) as rhsp,
                tc.tile_pool(name="scg", bufs=2) as scgp,
                tc.tile_pool(name="tree", bufs=2) as treep,
                tc.tile_pool(name="psc", bufs=2, space="PSUM") as psc,
            ):
                pairs = [(2 * i, 2 * i + 1 if 2 * i + 1 < NG else None)
                         for i in range((NG + 1) // 2)]
                for pi, (ga, gb) in enumerate(
                        [pr for _ in range(loop_reps) for pr in pairs]):
                    width = GRP if gb is None else 2 * GRP
                    npair = 1 if gb is None else 2
                    psum_direct = False
                    rhs = rhsp.tile([KF, width], F16)
                    for c0 in range(0, width, 2048):
                        nc.sync.dma_start(rhs[:, ds(c0, 2048)],
                                          XtA[:, ds(ga * GRP + c0, 2048)])
                    t1 = treep.tile([P, npair, 2048], F16, tag="t1")
                    if psum_direct:
                        # DVE consumes PSUM pairs directly (no ACT copies)
                        for g in range(npair):
                            psh = []
                            for h in range(2):
                                ps = psc.tile([P, 2048], F32)
                                for j0 in range(0, 2048, 512):
                                    nc.tensor.matmul(
                                        ps[:, ds(j0, 512)],
                                        lhsT=xT_t[:],
                                        rhs=rhs[:, ds((2 * g + h) * 2048 + j0,
                                                      512)],
                                        start=True, stop=True,
                                    )
                                psh.append(ps)
                            nc.vector.tensor_tensor(
                                t1[:, g, :], psh[0][:], psh[1][:],
                                op=mybir.AluOpType.max)
                    else:
                        scg = scgp.tile([P, width], F16)
                        for h in range(2 * npair):
                            ps = psc.tile([P, 2048], F32)
                            for j0 in range(0, 2048, 512):
                                nc.tensor.matmul(
                                    ps[:, ds(j0, 512)],
                                    lhsT=xT_t[:],
                                    rhs=rhs[:, ds(h * 2048 + j0, 512)],
                                    start=True, stop=True,
                                )
                            nc.scalar.copy(scg[:, ds(h * 2048, 2048)], ps[:])
                        sv = scg[:].rearrange("p (g h c) -> p g h c",
                                              g=npair, h=2)
                        nc.vector.tensor_tensor(
                            t1[:], sv[:, :, 0, :], sv[:, :, 1, :],
                            op=mybir.AluOpType.max)
                    tv = t1[:].rearrange("p g (h c) -> p g h c", h=2)
                    t2 = treep.tile([P, npair, 1024], F16, tag="t2")
                    nc.vector.tensor_tensor(
                        t2[:], tv[:, :, 0, :], tv[:, :, 1, :],
                        op=mybir.AluOpType.max)
                    tv2 = t2[:].rearrange("p g (h c) -> p g h c", h=2)
                    bm = treep.tile([P, npair, COSET], F16, tag="bm")
                    nc.vector.tensor_tensor(
                        bm[:], tv2[:, :, 0, :], tv2[:, :, 1, :],
                        op=mybir.AluOpType.max)
                    pp = pi % NPAIR
                    bmf = bm[:].rearrange("p a b -> p (a b)")
                    nc.vector.max(cand_v16[:, ts(pp, 8)], bmf)
                    nc.vector.max_index(
                        cand_pos[:, ts(pp, 8)], cand_v16[:, ts(pp, 8)], bmf)
                    if pi == 1:
                        load_tail_consts()

            # ---- phases 2+3+4: pipelined merge/gather/rescore/gate ----
            candPf = const.tile([P, NCAND], F32)
            nc.vector.tensor_copy(candPf[:], cand_v16[:])
            nc.vector.tensor_tensor(
                candPf[:].bitcast(U32), candPf[:].bitcast(U32), slot_u[:],
                op=mybir.AluOpType.bitwise_or)
            # candidate -> coset-block index (g*512 + bmpos), as f32
            cposf = const.tile([P, NCAND], F32)
            nc.vector.tensor_copy(cposf[:], cand_pos[:])
            cblkf = const.tile([P, NCAND], F32)
            nc.vector.tensor_tensor(cblkf[:], gblkf[:], cposf[:],
                                    op=mybir.AluOpType.add)

            wvalP = const.tile([P, K], F32)
            wslot_u = const.tile([P, K], U32)
            wslotf = const.tile([P, K], F32)
            wblkf = const.tile([P, K], F32)
            wblk = const.tile([P, K], I32)
            stt_s = const.tile([P, NCAND], F32)
            s8 = const.tile([P, K, 8], F32)
            rmax = const.tile([P, K], F32)
            nfT = const.tile([FWG, K * P], F16)
            gatedT = const.tile([C, K * P], F32)
            aggR = const.tile([C, P, 4], F32)

            with (
                tc.tile_pool(name="psm", bufs=2, space="PSUM") as psm,
                tc.tile_pool(name="resc", bufs=2) as rescp,
            ):
                # phase A: all 4 merge rounds + coset gathers issued up front
                nf8s = []
                for r in range(4):
                    r8 = ts(r, 8)
                    nc.vector.max(wvalP[:, r8], candPf[:])
                    if r < 3:
                        nc.vector.match_replace(
                            candPf[:], wvalP[:, r8], candPf[:], imm_value=NEG)
                    nc.vector.tensor_tensor(
                        wslot_u[:, r8], wvalP[:, r8].bitcast(U32), mask9[:],
                        op=mybir.AluOpType.bitwise_and)
                    nc.vector.tensor_copy(wslotf[:, r8], wslot_u[:, r8])
                    # wblk[k] = cblkf[slot == wslot[k]] via STT accumulate;
                    # each winner's coset-block gather issues right after its
                    # extraction so Pool streams while DVE extracts the rest
                    nf8 = rescp.tile([P, 8, 8 * FWG], F16, tag="nf8", bufs=4)
                    nf8s.append(nf8)
                    for k in range(r * 8, r * 8 + 8):
                        nc.vector.scalar_tensor_tensor(
                            out=stt_s[:],
                            in0=slotf[:],
                            scalar=wslotf[:, k:k + 1],
                            in1=cblkf[:],
                            op0=mybir.AluOpType.is_equal,
                            op1=mybir.AluOpType.mult,
                            accum_out=wblkf[:, k:k + 1],
                        )
                        nc.vector.tensor_copy(wblk[:, k:k + 1],
                                              wblkf[:, k:k + 1])
                        nc.gpsimd.indirect_dma_start(
                            out=nf8[:, k - r * 8, :],
                            out_offset=None,
                            in_=XrC,
                            in_offset=bass.IndirectOffsetOnAxis(
                                ap=wblk[:, k:k + 1], axis=0),
                        )
                # phase B: rescore/select/transpose/gate per round
                for r in range(4):
                    r8 = ts(r, 8)
                    nf8 = nf8s[r]
                    # rescore: s8[w, j] = sum_f nf8[w, j, f] * xRe[f]
                    prodr = rescp.tile([P, 64, FWG], F16, tag="prod")
                    nc.vector.tensor_tensor(
                        prodr[:],
                        nf8[:].rearrange("p w (j f) -> p (w j) f", f=FWG),
                        xRe_t[:].unsqueeze(1).broadcast_to([P, 64, FWG]),
                        op=mybir.AluOpType.mult)
                    foldr = rescp.tile([P, 64, 34], F16, tag="fold")
                    nc.vector.tensor_tensor(
                        foldr[:], prodr[:, :, 0:34],
                        prodr[:, :, 34:68], op=mybir.AluOpType.add)
                    foldr2 = rescp.tile([P, 64, 17], F16, tag="fold2")
                    nc.vector.tensor_tensor(
                        foldr2[:], foldr[:, :, 0:17],
                        foldr[:, :, 17:34], op=mybir.AluOpType.add)
                    nc.vector.reduce_sum(
                        s8[:, r8, :],
                        foldr2[:].rearrange("p (w j) c -> p w j c", j=8),
                        axis=mybir.AxisListType.X)
                    nc.vector.reduce_max(rmax[:, r8], s8[:, r8, :],
                                         axis=mybir.AxisListType.X)
                    # exclusive argmax mask over j: mask*(1..8), == its max
                    eqj = rescp.tile([P, 8, 8], F32, tag="eqj")
                    nc.vector.tensor_tensor(
                        eqj[:], s8[:, r8, :],
                        rmax[:, r8].unsqueeze(2).broadcast_to([P, 8, 8]),
                        op=mybir.AluOpType.is_ge)
                    nc.vector.tensor_tensor(eqj[:], eqj[:], jconst[:],
                                            op=mybir.AluOpType.mult)
                    jmx = rescp.tile([P, 8], F32, tag="jmx")
                    nc.vector.reduce_max(jmx[:], eqj[:],
                                         axis=mybir.AxisListType.X)
                    msel = rescp.tile([P, 8, 8], F16, tag="msel")
                    nc.vector.tensor_tensor(
                        msel[:], eqj[:],
                        jmx[:].unsqueeze(2).broadcast_to([P, 8, 8]),
                        op=mybir.AluOpType.is_equal)
                    # select the argmax row: mask rows then fold over j
                    mrow = rescp.tile([P, 64, FWG], F16, tag="mrow")
                    nc.vector.tensor_tensor(
                        mrow[:],
                        nf8[:].rearrange("p w (j f) -> p (w j) f", f=FWG),
                        msel[:].rearrange("p w j -> p (w j)")
                        .unsqueeze(2).broadcast_to([P, 64, FWG]),
                        op=mybir.AluOpType.mult)
                    mv = mrow[:].rearrange("p (w j) f -> p w j f", j=8)
                    f4 = rescp.tile([P, 8, 4, FWG], F16, tag="f4")
                    nc.vector.tensor_tensor(f4[:], mv[:, :, 0:4, :],
                                            mv[:, :, 4:8, :],
                                            op=mybir.AluOpType.add)
                    f2 = rescp.tile([P, 8, 2, FWG], F16, tag="f2")
                    nc.vector.tensor_tensor(f2[:], f4[:, :, 0:2, :],
                                            f4[:, :, 2:4, :],
                                            op=mybir.AluOpType.add)
                    sel = rescp.tile([P, 8, FWG], F16, tag="sel")
                    nc.vector.tensor_tensor(
                        sel[:], f2[:, :, 0, :], f2[:, :, 1, :],
                        op=mybir.AluOpType.add)
                    # transposes + gate for this round's 8 winners
                    for half in range(2):
                        pt = psm.tile([FWG, 4 * P], F16, tag="pt")
                        for i in range(4):
                            w = half * 4 + i
                            nc.tensor.transpose(pt[:, ds(i * P, P)],
                                                sel[:, w, :], ident16[:])
                        nc.scalar.copy(
                            nfT[:, ds((r * 8 + half * 4) * P, 4 * P)], pt[:])
                    for half in range(2):
                        j = 2 * r + half
                        gp = psm.tile([C, 512], F32, tag="gp")
                        nc.tensor.matmul(gp[:], lhsT=Wg_t[:],
                                         rhs=nfT[:, ts(j, 512)],
                                         start=True, stop=True)
                        nc.scalar.activation(
                            gatedT[:, ts(j, 512)], gp[:],
                            mybir.ActivationFunctionType.Tanh, bias=bg_t[:])
                    # partial neighbor-sum for this round's 8 winners
                    nc.vector.reduce_sum(
                        aggR[:, :, r], gatedT[:, ds(r * 8 * P, 8 * P)]
                        .rearrange("c (k q) -> c q k", k=8),
                        axis=mybir.AxisListType.X)

                aggT = const.tile([C, P], F32)
                nc.vector.reduce_sum(aggT[:], aggR[:],
                                     axis=mybir.AxisListType.X)
                oc = const.tile([F + C, P], F32)
                nc.vector.tensor_copy(oc[0:F, :], xT32_t[:])
                nc.vector.tensor_copy(oc[F:F + C, :], aggT[:])
                h1p = psm.tile([H, P], F32, tag="h1p")
                nc.tensor.matmul(h1p[:], lhsT=W1_t[:], rhs=oc[:],
                                 start=True, stop=True)
                h1 = const.tile([H, P], F32)
                nc.scalar.activation(h1[:], h1p[:],
                                     mybir.ActivationFunctionType.Tanh,
                                     bias=b1_t[:])
                op_ = psm.tile([1, P], F32, tag="op")
                nc.tensor.matmul(op_[:], lhsT=Wl_t[:], rhs=h1[:],
                                 start=True, stop=True)
                outt = const.tile([1, P], F32)
                nc.scalar.activation(outt[:], op_[:],
                                     mybir.ActivationFunctionType.Sigmoid,
                                     bias=bl_t[:])
                nc.sync.dma_start(out, outt[:])

    nc.compile()
    return nc


def prep_inputs(x, X_data, y, W_gate, b_gate, W1, b1, W_last, b_last):
    x = np.asarray(x, np.float32)
    X_data = np.asarray(X_data, np.float32)
    y = np.asarray(y, np.float32)
    n = len(X_data)

    norm = (-0.5 * (X_data * X_data).sum(1)).astype(np.float32)
    norm_hi = norm.astype(np.float16)
    norm_lo = (norm - norm_hi.astype(np.float32)).astype(np.float16)

    XtA = np.zeros((KF, NPAD), np.float16)
    XtA[:F, :n] = X_data.T.astype(np.float16)
    XtA[F, :n] = norm_hi
    XtA[F + 1, :n] = norm_lo
    XtA[F, n:] = np.float16(PADV)

    # coset-block row table: XrC[g*512 + b, j] = row(col g*4096 + 512j + b),
    # row = [X(64) | norm | y | 0 | 0] fp16
    V = np.zeros((NPAD, FWG), np.float16)
    V[:n, :F] = X_data.astype(np.float16)
    V[:n, F] = norm.astype(np.float16)
    V[:n, F + 1] = y.astype(np.float16)
    V[n:, F] = np.float16(PADV)
    XrC = (V.reshape(NG, 8, COSET, FWG)
           .transpose(0, 2, 1, 3)
           .reshape(NBLK, 8 * FWG))

    # gate weights in block-row order: X rows, norm row -> 0, y row
    Wgp = np.zeros((FWG, C), np.float16)
    Wgp[:F] = np.asarray(W_gate, np.float32)[:F].astype(np.float16)
    Wgp[F + 1] = np.asarray(W_gate, np.float32)[F].astype(np.float16)

    shared = {
        "XtA": XtA,
        "XrC": XrC,
        "Wg": Wgp,
        "W1": np.asarray(W1, np.float32),
        "Wl": np.asarray(W_last, np.float32).reshape(H, 1),
        "bg": np.asarray(b_gate, np.float32).reshape(C, 1),
        "b1": np.asarray(b1, np.float32).reshape(H, 1),
        "bl": np.asarray(b_last, np.float32).reshape(1, 1),
    }
    in_maps = []
    for c in range(CORES):
        xc = x[c * QPC:(c + 1) * QPC]
        xTa = np.ones((KF, QPC), np.float16)
        xTa[:F] = xc.T.astype(np.float16)
        xre = np.zeros((QPC, FWG), np.float16)
        xre[:, :F] = xc.astype(np.float16)
        xre[:, F] = 1.0
        m = dict(shared)
        m["xT"] = xTa
        m["xT32"] = xc.T.copy()
        m["xRe"] = xre
        in_maps.append(m)
    return in_maps


_NC_CACHE = {}


def _get_program():
    if "nc" not in _NC_CACHE:
        _NC_CACHE["nc"] = build_program()
    return _NC_CACHE["nc"]


def kernel(x, X_data, y, W_gate, b_gate, W1, b1, W_last, b_last):
    from concourse import bass_utils

    nc = _get_program()
    in_maps = prep_inputs(x, X_data, y, W_gate, b_gate, W1, b1, W_last, b_last)
    res = bass_utils.run_bass_kernel_spmd(
        nc, in_maps, core_ids=list(range(CORES))
    )
    outs = [res.results[c]["out"].reshape(QPC) for c in range(CORES)]
    return np.concatenate(outs).reshape(B, 1).astype(np.float32)

